# revision 10
# baseline (speedup 1.0000x reference)
"""Trainium2 Bass kernel for nn_DGG_StraightThrough.

The reference's pairwise-logit MLP is mathematically dead: softmax over the
singleton feature dim is identically 1, so log_p == 0 and the gumbel logits
y equal `temp` exactly (bit-for-bit, verified).  The output is therefore the
straight-through top-8 row indicator of temp, identical for every batch
entry:  adj[b,i,j] = 1.0 iff temp[i,j] is among the 8 largest of row i.

Sharding: row-parallel over N=2048 across 8 cores (256 rows each).  Each
core: DMA its [256,2048] slab in, DVE Max8 for the per-row 8th-largest
threshold, is_ge compare to build the 0/1 mask, DMA the mask out.  Host
concatenates the 8 slabs and broadcasts over B=4.
"""

import sys

import numpy as np

if "/opt/trn_rl_repo" not in sys.path:
    sys.path.insert(0, "/opt/trn_rl_repo")

B, N, K = 4, 2048, 8
N_CORES = 8
ROWS = N // N_CORES  # 256 rows per core
P = 128  # SBUF partitions

# Hooks for a driving harness (test.py): extra kwargs for run_bass_kernel_spmd
# and the last BassKernelResults (exec_time_ns etc).
RUN_KWARGS: dict = {}
LAST_RESULT = None

_PROGRAM = None


def _build_program():
    import concourse.bass as bass
    import concourse.mybir as mybir

    nc = bass.Bass()
    t_in = nc.declare_dram_parameter("t", [ROWS, N], mybir.dt.float32, isOutput=False)
    # u8 wire format for the 0/1 mask (lossless); host casts back to f32
    out = nc.declare_dram_parameter("out", [ROWS, N], mybir.dt.uint8, isOutput=True)

    nblk = ROWS // P  # 128-row chunks, pipelined in -> max -> cmp -> out

    with (
        nc.sbuf_tensor([P, nblk * N], mybir.dt.float32) as tile,
        nc.sbuf_tensor([P, nblk * N], mybir.dt.uint8) as mask,
        nc.sbuf_tensor([P, 8 * nblk], mybir.dt.float32) as top8,
        # per-chunk DMA sems: HWDGE transfers on different queues can
        # complete out of order, so one shared counting sem would race
        nc.semaphore("in_sem0") as in_sem0,
        nc.semaphore("in_sem1") as in_sem1,
        nc.semaphore("out_sem") as out_sem,
        nc.semaphore("mx_sem") as mx_sem,
        nc.semaphore("cp_sem") as cp_sem,
        nc.Block() as block,
    ):
        in_sems = [in_sem0, in_sem1]
        assert nblk == 2

        @block.scalar
        def _(scalar):
            # ACT's own HWDGE ring (qActDynamicHW): issues in0 ~1us before the
            # Sync engine clears its preamble drain/sem-init backlog
            scalar.dma_start(out=tile[:, 0:N], in_=t_in[0:P, :]).then_inc(
                in_sems[0], 16
            )

        @block.sync
        def _(sync):
            sync.dma_start(
                out=tile[:, N : 2 * N], in_=t_in[P : 2 * P, :]
            ).then_inc(in_sems[1], 16)
            for b in range(nblk):
                sync.wait_ge(cp_sem, b + 1)
                sync.dma_start(
                    out=out[b * P : (b + 1) * P, :], in_=mask[:, b * N : (b + 1) * N]
                ).then_inc(out_sem, 16)
            sync.wait_ge(out_sem, 16 * nblk)

        @block.vector
        def _(vector):
            # DVE does max+cmp per chunk (GpSimd tensor_scalar is ~24x slower;
            # ACT has no exact step fn).  cmp_b right after max_b so the out-DMA
            # of chunk b overlaps with chunk b+1's compute.
            for b in range(nblk):
                vector.wait_ge(in_sems[b], 16)
                vector.max(
                    top8[:, 8 * b : 8 * (b + 1)], tile[:, b * N : (b + 1) * N]
                ).then_inc(mx_sem, 1)
                # sem hop: the scalar-ptr operand of tensor_scalar is fetched
                # early, racing the in-pipeline MAX8 write on the same engine
                vector.wait_ge(mx_sem, b + 1)
                # mask = (t >= 8th largest of its row) -> 1.0 / 0.0
                vector.tensor_scalar(
                    mask[:, b * N : (b + 1) * N],
                    tile[:, b * N : (b + 1) * N],
                    top8[:, 8 * b + 7 : 8 * b + 8],
                    None,
                    mybir.AluOpType.is_ge,
                ).then_inc(cp_sem, 1)
    return nc


def kernel(**inputs: np.ndarray) -> np.ndarray:
    global _PROGRAM, LAST_RESULT
    from concourse.bass_utils import run_bass_kernel_spmd

    temp = np.ascontiguousarray(np.asarray(inputs["temp"], dtype=np.float32))
    assert temp.shape == (N, N)

    if _PROGRAM is None:
        _PROGRAM = _build_program()

    in_maps = [
        {"t": np.ascontiguousarray(temp[c * ROWS : (c + 1) * ROWS])}
        for c in range(N_CORES)
    ]
    res = run_bass_kernel_spmd(_PROGRAM, in_maps, list(range(N_CORES)), **RUN_KWARGS)
    LAST_RESULT = res

    mask = np.concatenate([res.results[c]["out"] for c in range(N_CORES)], axis=0)
    mask = mask.astype(np.float32)
    return np.ascontiguousarray(np.broadcast_to(mask[None], (B, N, N)))


# revision 11
# speedup vs baseline: 1.1012x; 1.1012x over previous
"""Trainium2 Bass kernel for nn_DGG_StraightThrough.

The reference's pairwise-logit MLP is mathematically dead: softmax over the
singleton feature dim is identically 1, so log_p == 0 and the gumbel logits
y equal `temp` exactly (bit-for-bit, verified).  The output is therefore the
straight-through top-8 row indicator of temp, identical for every batch
entry:  adj[b,i,j] = 1.0 iff temp[i,j] is among the 8 largest of row i.

Sharding: row-parallel over N=2048 across 8 cores (256 rows each).  Each
core: DMA its [256,2048] slab in, DVE Max8 for the per-row 8th-largest
threshold, is_ge compare to build the 0/1 mask, DMA the mask out.  Host
concatenates the 8 slabs and broadcasts over B=4.
"""

import sys

import numpy as np

if "/opt/trn_rl_repo" not in sys.path:
    sys.path.insert(0, "/opt/trn_rl_repo")

B, N, K = 4, 2048, 8
N_CORES = 8
ROWS = N // N_CORES  # 256 rows per core
P = 128  # SBUF partitions

# Hooks for a driving harness (test.py): extra kwargs for run_bass_kernel_spmd
# and the last BassKernelResults (exec_time_ns etc).
RUN_KWARGS: dict = {}
LAST_RESULT = None

_PROGRAM = None


def _build_program():
    import concourse.bass as bass
    import concourse.mybir as mybir

    nc = bass.Bass()
    t_in = nc.declare_dram_parameter("t", [ROWS, N], mybir.dt.float32, isOutput=False)
    # u8 wire format for the 0/1 mask (lossless); host casts back to f32
    out = nc.declare_dram_parameter("out", [ROWS, N], mybir.dt.uint8, isOutput=True)

    nblk = ROWS // P  # 128-row chunks, pipelined in -> max -> cmp -> out

    with (
        nc.sbuf_tensor([P, nblk * N], mybir.dt.float32) as tile,
        nc.sbuf_tensor([P, nblk * N], mybir.dt.uint8) as mask,
        nc.sbuf_tensor([P, 8 * nblk], mybir.dt.float32) as top8,
        # per-chunk in-DMA sems: HWDGE transfers on different queues can
        # complete out of order, so one shared counting sem would race
        nc.semaphore("in_sem0") as in_sem0,
        nc.semaphore("in_sem1") as in_sem1,
        nc.semaphore("out_sem") as out_sem,
        nc.semaphore("v_sem") as v_sem,
        nc.Block() as block,
    ):
        in_sems = [in_sem0, in_sem1]
        assert nblk == 2

        @block.sync
        def _(sync):
            # in0 first and alone: a concurrently-issued in1 would share HBM
            # bandwidth and delay in0, which gates the whole DVE chain
            for b in range(nblk):
                sync.dma_start(
                    out=tile[:, b * N : (b + 1) * N], in_=t_in[b * P : (b + 1) * P, :]
                ).then_inc(in_sems[b], 16)
            for b in range(nblk):
                sync.wait_ge(v_sem, 2 * (b + 1))
                sync.dma_start(
                    out=out[b * P : (b + 1) * P, :], in_=mask[:, b * N : (b + 1) * N]
                ).then_inc(out_sem, 16)
            sync.wait_ge(out_sem, 16 * nblk)

        @block.vector
        def _(vector):
            # DVE does max+cmp per chunk (GpSimd tensor_scalar is ~24x slower;
            # ACT has no exact step fn).  cmp_b right after max_b so the out-DMA
            # of chunk b overlaps with chunk b+1's compute.  One sem counts all
            # DVE ops: max_b -> 2b+1, cmp_b -> 2b+2 (in-order engine).
            for b in range(nblk):
                vector.wait_ge(in_sems[b], 16)
                vector.max(
                    top8[:, 8 * b : 8 * (b + 1)], tile[:, b * N : (b + 1) * N]
                ).then_inc(v_sem, 1)
                # sem hop: the scalar-ptr operand of tensor_scalar is fetched
                # early, racing the in-pipeline MAX8 write on the same engine
                vector.wait_ge(v_sem, 2 * b + 1)
                # mask = (t >= 8th largest of its row) -> 1.0 / 0.0
                vector.tensor_scalar(
                    mask[:, b * N : (b + 1) * N],
                    tile[:, b * N : (b + 1) * N],
                    top8[:, 8 * b + 7 : 8 * b + 8],
                    None,
                    mybir.AluOpType.is_ge,
                ).then_inc(v_sem, 1)
    return nc


def kernel(**inputs: np.ndarray) -> np.ndarray:
    global _PROGRAM, LAST_RESULT
    from concourse.bass_utils import run_bass_kernel_spmd

    temp = np.ascontiguousarray(np.asarray(inputs["temp"], dtype=np.float32))
    assert temp.shape == (N, N)

    if _PROGRAM is None:
        _PROGRAM = _build_program()

    in_maps = [
        {"t": np.ascontiguousarray(temp[c * ROWS : (c + 1) * ROWS])}
        for c in range(N_CORES)
    ]
    res = run_bass_kernel_spmd(_PROGRAM, in_maps, list(range(N_CORES)), **RUN_KWARGS)
    LAST_RESULT = res

    mask = np.concatenate([res.results[c]["out"] for c in range(N_CORES)], axis=0)
    mask = mask.astype(np.float32)
    return np.ascontiguousarray(np.broadcast_to(mask[None], (B, N, N)))


# revision 12
# speedup vs baseline: 1.1057x; 1.0041x over previous
"""Trainium2 Bass kernel for nn_DGG_StraightThrough.

The reference's pairwise-logit MLP is mathematically dead: softmax over the
singleton feature dim is identically 1, so log_p == 0 and the gumbel logits
y equal `temp` exactly (bit-for-bit, verified).  The output is therefore the
straight-through top-8 row indicator of temp, identical for every batch
entry:  adj[b,i,j] = 1.0 iff temp[i,j] is among the 8 largest of row i.

Sharding: row-parallel over N=2048 across 8 cores (256 rows each).  Each
core: DMA its [256,2048] slab in, DVE Max8 for the per-row 8th-largest
threshold, is_ge compare to build the 0/1 mask, DMA the mask out.  Host
concatenates the 8 slabs and broadcasts over B=4.
"""

import sys

import numpy as np

if "/opt/trn_rl_repo" not in sys.path:
    sys.path.insert(0, "/opt/trn_rl_repo")

B, N, K = 4, 2048, 8
N_CORES = 8
ROWS = N // N_CORES  # 256 rows per core
P = 128  # SBUF partitions

# Hooks for a driving harness (test.py): extra kwargs for run_bass_kernel_spmd
# and the last BassKernelResults (exec_time_ns etc).
RUN_KWARGS: dict = {}
LAST_RESULT = None

_PROGRAM = None


def _build_program():
    import concourse.bass as bass
    import concourse.mybir as mybir

    nc = bass.Bass(enable_partition_id=False, monotonic_sem_count=0)
    t_in = nc.declare_dram_parameter("t", [ROWS, N], mybir.dt.float32, isOutput=False)
    # u8 wire format for the 0/1 mask (lossless); host casts back to f32
    out = nc.declare_dram_parameter("out", [ROWS, N], mybir.dt.uint8, isOutput=True)

    nblk = ROWS // P  # 128-row chunks, pipelined in -> max -> cmp -> out

    with (
        nc.sbuf_tensor([P, nblk * N], mybir.dt.float32) as tile,
        nc.sbuf_tensor([P, nblk * N], mybir.dt.uint8) as mask,
        nc.sbuf_tensor([P, 8 * nblk], mybir.dt.float32) as top8,
        # per-chunk in-DMA sems: HWDGE transfers on different queues can
        # complete out of order, so one shared counting sem would race
        nc.semaphore("in_sem0") as in_sem0,
        nc.semaphore("in_sem1") as in_sem1,
        nc.semaphore("out_sem") as out_sem,
        nc.semaphore("v_sem") as v_sem,
        nc.Block() as block,
    ):
        in_sems = [in_sem0, in_sem1]
        assert nblk == 2

        @block.sync
        def _(sync):
            # in0 first and alone: a concurrently-issued in1 would share HBM
            # bandwidth and delay in0, which gates the whole DVE chain
            for b in range(nblk):
                sync.dma_start(
                    out=tile[:, b * N : (b + 1) * N], in_=t_in[b * P : (b + 1) * P, :]
                ).then_inc(in_sems[b], 16)
            for b in range(nblk):
                sync.wait_ge(v_sem, 2 * (b + 1))
                sync.dma_start(
                    out=out[b * P : (b + 1) * P, :], in_=mask[:, b * N : (b + 1) * N]
                ).then_inc(out_sem, 16)
            sync.wait_ge(out_sem, 16 * nblk)

        @block.vector
        def _(vector):
            # DVE does max+cmp per chunk (GpSimd tensor_scalar is ~24x slower;
            # ACT has no exact step fn).  cmp_b right after max_b so the out-DMA
            # of chunk b overlaps with chunk b+1's compute.  One sem counts all
            # DVE ops: max_b -> 2b+1, cmp_b -> 2b+2 (in-order engine).
            for b in range(nblk):
                vector.wait_ge(in_sems[b], 16)
                vector.max(
                    top8[:, 8 * b : 8 * (b + 1)], tile[:, b * N : (b + 1) * N]
                ).then_inc(v_sem, 1)
                # sem hop: the scalar-ptr operand of tensor_scalar is fetched
                # early, racing the in-pipeline MAX8 write on the same engine
                vector.wait_ge(v_sem, 2 * b + 1)
                # mask = (t >= 8th largest of its row) -> 1.0 / 0.0
                vector.tensor_scalar(
                    mask[:, b * N : (b + 1) * N],
                    tile[:, b * N : (b + 1) * N],
                    top8[:, 8 * b + 7 : 8 * b + 8],
                    None,
                    mybir.AluOpType.is_ge,
                ).then_inc(v_sem, 1)
    return nc


def kernel(**inputs: np.ndarray) -> np.ndarray:
    global _PROGRAM, LAST_RESULT
    from concourse.bass_utils import run_bass_kernel_spmd

    temp = np.ascontiguousarray(np.asarray(inputs["temp"], dtype=np.float32))
    assert temp.shape == (N, N)

    if _PROGRAM is None:
        _PROGRAM = _build_program()

    in_maps = [
        {"t": np.ascontiguousarray(temp[c * ROWS : (c + 1) * ROWS])}
        for c in range(N_CORES)
    ]
    res = run_bass_kernel_spmd(_PROGRAM, in_maps, list(range(N_CORES)), **RUN_KWARGS)
    LAST_RESULT = res

    mask = np.concatenate([res.results[c]["out"] for c in range(N_CORES)], axis=0)
    mask = mask.astype(np.float32)
    return np.ascontiguousarray(np.broadcast_to(mask[None], (B, N, N)))


# revision 14
# speedup vs baseline: 1.1324x; 1.0242x over previous
"""Trainium2 Bass kernel for nn_DGG_StraightThrough.

The reference's pairwise-logit MLP is mathematically dead: softmax over the
singleton feature dim is identically 1, so log_p == 0 and the gumbel logits
y equal `temp` exactly (bit-for-bit, verified).  The output is therefore the
straight-through top-8 row indicator of temp, identical for every batch
entry:  adj[b,i,j] = 1.0 iff temp[i,j] is among the 8 largest of row i.

Sharding: row-parallel over N=2048 across 8 cores (256 rows each).  Each
core: DMA its [256,2048] slab in, DVE Max8 for the per-row 8th-largest
threshold, is_ge compare to build the 0/1 mask, DMA the mask out.  Host
concatenates the 8 slabs and broadcasts over B=4.
"""

import sys

import numpy as np

if "/opt/trn_rl_repo" not in sys.path:
    sys.path.insert(0, "/opt/trn_rl_repo")

B, N, K = 4, 2048, 8
N_CORES = 8
ROWS = N // N_CORES  # 256 rows per core
P = 128  # SBUF partitions

# Hooks for a driving harness (test.py): extra kwargs for run_bass_kernel_spmd
# and the last BassKernelResults (exec_time_ns etc).
RUN_KWARGS: dict = {}
LAST_RESULT = None

_PROGRAM = None


def _build_program():
    import concourse.bass as bass
    import concourse.mybir as mybir

    class _LeanBass(bass.Bass):
        # Skip the barrier Bass.__init__ emits after const-AP registration:
        # this kernel never reads const APs, Sync's DGE table load precedes
        # its DMAs in program order, and the NRT entry pseudo-barrier already
        # orders the gpsimd sem-clears.  Saves ~1us of preamble.
        _skip_init_barrier = False

        def all_engine_barrier(self, **kw):
            if _LeanBass._skip_init_barrier:
                return
            return super().all_engine_barrier(**kw)

    _LeanBass._skip_init_barrier = True
    try:
        nc = _LeanBass(enable_partition_id=False, monotonic_sem_count=0)
    finally:
        _LeanBass._skip_init_barrier = False
    t_in = nc.declare_dram_parameter("t", [ROWS, N], mybir.dt.float32, isOutput=False)
    # u8 wire format for the 0/1 mask (lossless); host casts back to f32
    out = nc.declare_dram_parameter("out", [ROWS, N], mybir.dt.uint8, isOutput=True)

    nblk = ROWS // P  # 128-row chunks, pipelined in -> max -> cmp -> out

    with (
        nc.sbuf_tensor([P, nblk * N], mybir.dt.float32) as tile,
        nc.sbuf_tensor([P, nblk * N], mybir.dt.uint8) as mask,
        nc.sbuf_tensor([P, 8 * nblk], mybir.dt.float32) as top8,
        # per-chunk in-DMA sems: HWDGE transfers on different queues can
        # complete out of order, so one shared counting sem would race
        nc.semaphore("in_sem0") as in_sem0,
        nc.semaphore("in_sem1") as in_sem1,
        nc.semaphore("out_sem") as out_sem,
        nc.semaphore("v_sem") as v_sem,
        # no SWDGE DMAs issued -> skip GpSimd's expensive dge_drain at exit
        nc.Block(no_gpsimd_drain=True) as block,
    ):
        in_sems = [in_sem0, in_sem1]
        assert nblk == 2

        @block.sync
        def _(sync):
            # in0 first and alone: a concurrently-issued in1 would share HBM
            # bandwidth and delay in0, which gates the whole DVE chain
            for b in range(nblk):
                sync.dma_start(
                    out=tile[:, b * N : (b + 1) * N], in_=t_in[b * P : (b + 1) * P, :]
                ).then_inc(in_sems[b], 16)
            for b in range(nblk):
                sync.wait_ge(v_sem, 2 * (b + 1))
                sync.dma_start(
                    out=out[b * P : (b + 1) * P, :], in_=mask[:, b * N : (b + 1) * N]
                ).then_inc(out_sem, 16)
            sync.wait_ge(out_sem, 16 * nblk)

        @block.vector
        def _(vector):
            # DVE does max+cmp per chunk (GpSimd tensor_scalar is ~24x slower;
            # ACT has no exact step fn).  cmp_b right after max_b so the out-DMA
            # of chunk b overlaps with chunk b+1's compute.  One sem counts all
            # DVE ops: max_b -> 2b+1, cmp_b -> 2b+2 (in-order engine).
            for b in range(nblk):
                vector.wait_ge(in_sems[b], 16)
                vector.max(
                    top8[:, 8 * b : 8 * (b + 1)], tile[:, b * N : (b + 1) * N]
                ).then_inc(v_sem, 1)
                # sem hop: the scalar-ptr operand of tensor_scalar is fetched
                # early, racing the in-pipeline MAX8 write on the same engine
                vector.wait_ge(v_sem, 2 * b + 1)
                # mask = (t >= 8th largest of its row) -> 1.0 / 0.0
                vector.tensor_scalar(
                    mask[:, b * N : (b + 1) * N],
                    tile[:, b * N : (b + 1) * N],
                    top8[:, 8 * b + 7 : 8 * b + 8],
                    None,
                    mybir.AluOpType.is_ge,
                ).then_inc(v_sem, 1)
    return nc


def kernel(**inputs: np.ndarray) -> np.ndarray:
    global _PROGRAM, LAST_RESULT
    from concourse.bass_utils import run_bass_kernel_spmd

    temp = np.ascontiguousarray(np.asarray(inputs["temp"], dtype=np.float32))
    assert temp.shape == (N, N)

    if _PROGRAM is None:
        _PROGRAM = _build_program()

    in_maps = [
        {"t": np.ascontiguousarray(temp[c * ROWS : (c + 1) * ROWS])}
        for c in range(N_CORES)
    ]
    res = run_bass_kernel_spmd(_PROGRAM, in_maps, list(range(N_CORES)), **RUN_KWARGS)
    LAST_RESULT = res

    mask = np.concatenate([res.results[c]["out"] for c in range(N_CORES)], axis=0)
    mask = mask.astype(np.float32)
    return np.ascontiguousarray(np.broadcast_to(mask[None], (B, N, N)))


# revision 16
# speedup vs baseline: 1.1585x; 1.0230x over previous
"""Trainium2 Bass kernel for nn_DGG_StraightThrough.

The reference's pairwise-logit MLP is mathematically dead: softmax over the
singleton feature dim is identically 1, so log_p == 0 and the gumbel logits
y equal `temp` exactly (bit-for-bit, verified).  The output is therefore the
straight-through top-8 row indicator of temp, identical for every batch
entry:  adj[b,i,j] = 1.0 iff temp[i,j] is among the 8 largest of row i.

Sharding: row-parallel over N=2048 across 8 cores (256 rows each).  Each
core: DMA its [256,2048] slab in, DVE Max8 for the per-row 8th-largest
threshold, is_ge compare to build the 0/1 mask, DMA the mask out.  Host
concatenates the 8 slabs and broadcasts over B=4.
"""

import sys

import numpy as np

if "/opt/trn_rl_repo" not in sys.path:
    sys.path.insert(0, "/opt/trn_rl_repo")

B, N, K = 4, 2048, 8
N_CORES = 8
ROWS = N // N_CORES  # 256 rows per core
P = 128  # SBUF partitions

# Hooks for a driving harness (test.py): extra kwargs for run_bass_kernel_spmd
# and the last BassKernelResults (exec_time_ns etc).
RUN_KWARGS: dict = {}
LAST_RESULT = None

_PROGRAM = None


def _build_program():
    import concourse.bass as bass
    import concourse.mybir as mybir

    class _LeanBass(bass.Bass):
        # Skip the barrier Bass.__init__ emits after const-AP registration:
        # this kernel never reads const APs, Sync's DGE table load precedes
        # its DMAs in program order, and the NRT entry pseudo-barrier already
        # orders the gpsimd sem-clears.  Saves ~1us of preamble.
        _skip_init_barrier = False

        def all_engine_barrier(self, **kw):
            if _LeanBass._skip_init_barrier:
                return
            return super().all_engine_barrier(**kw)

    _LeanBass._skip_init_barrier = True
    try:
        nc = _LeanBass(enable_partition_id=False, monotonic_sem_count=0)
    finally:
        _LeanBass._skip_init_barrier = False
    t_in = nc.declare_dram_parameter("t", [ROWS, N], mybir.dt.float32, isOutput=False)
    # u8 wire format for the 0/1 mask (lossless); host casts back to f32
    out = nc.declare_dram_parameter("out", [ROWS, N], mybir.dt.uint8, isOutput=True)

    nblk = ROWS // P  # 128-row chunks, pipelined in -> max -> cmp -> out

    with (
        nc.sbuf_tensor([P, nblk * N], mybir.dt.float32) as tile,
        nc.sbuf_tensor([P, nblk * N], mybir.dt.uint8) as mask,
        nc.sbuf_tensor([P, 8 * nblk], mybir.dt.float32) as top8,
        # per-chunk in-DMA sems: HWDGE transfers on different queues can
        # complete out of order, so one shared counting sem would race
        nc.semaphore("in_sem0") as in_sem0,
        nc.semaphore("in_sem1") as in_sem1,
        nc.semaphore("out_sem") as out_sem,
        nc.semaphore("v_sem") as v_sem,
    ):
        in_sems = [in_sem0, in_sem1]
        assert nblk == 2

        # Issue the in-DMAs OUTSIDE the Block, directly after Sync's DGE-table
        # preamble: they depend on no other engine, so they need not wait for
        # the block-entry all-engine sync (~1.7us earlier start).  in0 first
        # and alone: a concurrently-issued in1 would share HBM bandwidth and
        # delay in0, which gates the whole DVE chain.
        for b in range(nblk):
            nc.sync.dma_start(
                out=tile[:, b * N : (b + 1) * N], in_=t_in[b * P : (b + 1) * P, :]
            ).then_inc(in_sems[b], 16)

        # no SWDGE DMAs issued -> skip GpSimd's expensive dge_drain at exit
        with nc.Block(no_gpsimd_drain=True) as block:

            @block.sync
            def _(sync):
                for b in range(nblk):
                    sync.wait_ge(v_sem, 2 * (b + 1))
                    sync.dma_start(
                        out=out[b * P : (b + 1) * P, :],
                        in_=mask[:, b * N : (b + 1) * N],
                    ).then_inc(out_sem, 16)
                sync.wait_ge(out_sem, 16 * nblk)

            @block.vector
            def _(vector):
                # DVE does max+cmp per chunk (GpSimd tensor_scalar is ~24x
                # slower; ACT has no exact step fn).  cmp_b right after max_b
                # so the out-DMA of chunk b overlaps with chunk b+1's compute.
                # One sem counts all DVE ops: max_b -> 2b+1, cmp_b -> 2b+2
                # (in-order engine).
                for b in range(nblk):
                    vector.wait_ge(in_sems[b], 16)
                    vector.max(
                        top8[:, 8 * b : 8 * (b + 1)], tile[:, b * N : (b + 1) * N]
                    ).then_inc(v_sem, 1)
                    # sem hop: the scalar-ptr operand of tensor_scalar is
                    # fetched early, racing the in-pipeline MAX8 write on the
                    # same engine
                    vector.wait_ge(v_sem, 2 * b + 1)
                    # mask = (t >= 8th largest of its row) -> 1.0 / 0.0
                    vector.tensor_scalar(
                        mask[:, b * N : (b + 1) * N],
                        tile[:, b * N : (b + 1) * N],
                        top8[:, 8 * b + 7 : 8 * b + 8],
                        None,
                        mybir.AluOpType.is_ge,
                    ).then_inc(v_sem, 1)
    return nc


def kernel(**inputs: np.ndarray) -> np.ndarray:
    global _PROGRAM, LAST_RESULT
    from concourse.bass_utils import run_bass_kernel_spmd

    temp = np.ascontiguousarray(np.asarray(inputs["temp"], dtype=np.float32))
    assert temp.shape == (N, N)

    if _PROGRAM is None:
        _PROGRAM = _build_program()

    in_maps = [
        {"t": np.ascontiguousarray(temp[c * ROWS : (c + 1) * ROWS])}
        for c in range(N_CORES)
    ]
    res = run_bass_kernel_spmd(_PROGRAM, in_maps, list(range(N_CORES)), **RUN_KWARGS)
    LAST_RESULT = res

    mask = np.concatenate([res.results[c]["out"] for c in range(N_CORES)], axis=0)
    mask = mask.astype(np.float32)
    return np.ascontiguousarray(np.broadcast_to(mask[None], (B, N, N)))
